# revision 9
# baseline (speedup 1.0000x reference)
"""TopK sparse autoencoder forward pass on 8 Trainium2 NeuronCores — v3.

Baseline structure (3-pass fp16 hi/lo encoder, threshold top-64 via per-256
chunk top-8 candidates + exact tau, dense masked fp16 decode) with all input
marshaling moved to the host: enc_weight arrives pre-transposed and hi/lo
fp16-split, x arrives bias-subtracted/transposed/split, dec_lookup arrives
fp16. The PE runs only encoder matmuls + mask transposes + decode matmuls.
"""

import sys

sys.path.insert(0, "/opt/trn_rl_repo")

import numpy as np  # noqa: E402

import concourse.bacc as bacc  # noqa: E402
import concourse.mybir as mybir  # noqa: E402
import concourse.tile as tile  # noqa: E402
from concourse.bass_utils import run_bass_kernel_spmd  # noqa: E402

dt = mybir.dt
Alu = mybir.AluOpType
Act = mybir.ActivationFunctionType

N_CORES = 8
E = 768
EC = E // 128  # 6 e-chunks
NEG_FILL = -1e30
G = 8  # decoder f-block accumulation group


def build_kernel(NB=4, NFB=48):
    """NB: batch tiles of 128 rows per core; NFB: feature blocks of 512."""
    B_loc = NB * 128
    F = NFB * 512
    G = min(globals()["G"], NFB)
    NCAND = NFB * 2 * 8  # top-8 per 256-feat chunk

    nc = bacc.Bacc("TRN2", target_bir_lowering=False, debug=False,
                   num_devices=N_CORES)
    xh_in = nc.dram_tensor("xTh", [E, B_loc], dt.float16, kind="ExternalInput").ap()
    xl_in = nc.dram_tensor("xTl", [E, B_loc], dt.float16, kind="ExternalInput").ap()
    wh_in = nc.dram_tensor("wTh", [E, F], dt.float16, kind="ExternalInput").ap()
    wl_in = nc.dram_tensor("wTl", [E, F], dt.float16, kind="ExternalInput").ap()
    dec_in = nc.dram_tensor("dec16", [F, E], dt.float16, kind="ExternalInput").ap()
    biasf_in = nc.dram_tensor("biasf", [128, E], dt.float32, kind="ExternalInput").ap()
    id16_in = nc.dram_tensor("ident16", [128, 128], dt.float16, kind="ExternalInput").ap()
    out_ext = nc.dram_tensor("out", [B_loc, E], dt.float32, kind="ExternalOutput").ap()
    proj_scr = nc.dram_tensor("proj_scr", [B_loc, F], dt.float32).ap()

    xh_v = xh_in.rearrange("(ec p) b -> p ec b", p=128)
    xl_v = xl_in.rearrange("(ec p) b -> p ec b", p=128)
    wh_v = wh_in.rearrange("(ec p) f -> p ec f", p=128)
    wl_v = wl_in.rearrange("(ec p) f -> p ec f", p=128)
    dec_v = dec_in.rearrange("(blk t p) e -> blk p t e", p=128, t=4)
    out_v = out_ext.rearrange("(bt p) e -> bt p e", p=128)

    with tile.TileContext(nc) as tc:
        with tc.tile_pool(name="persist", bufs=1) as pp:
            xTh = pp.tile([128, EC, B_loc], dt.float16, tag="xTh")
            xTl = pp.tile([128, EC, B_loc], dt.float16, tag="xTl")
            id16 = pp.tile([128, 128], dt.float16, tag="id16")
            bias_full = pp.tile([128, E], dt.float32, tag="bias_full")
            # candidates per batch-tile
            cands = [pp.tile([128, NCAND], dt.float32, tag=f"cand{bt}",
                             name=f"cand{bt}") for bt in range(NB)]
            # recon accumulator
            recon = pp.tile([128, NB, E], dt.float32, tag="recon")
            nc.vector.memset(recon[:], 0.0)
            taus = []

            def tau_find(bt):
                """exact 64th-largest of bt's candidates (destroys cands[bt])."""
                m8 = None
                for r in range(8):
                    m8 = pp.tile([128, 8], dt.float32, tag=f"m8_{bt}_{r}",
                                 name=f"m8_{bt}_{r}")
                    nc.vector.max(m8[:], cands[bt][:])
                    if r < 7:
                        nc.vector.match_replace(cands[bt][:], m8[:], cands[bt][:],
                                                NEG_FILL)
                return m8

            # ---------------- Phase 1: encoder + candidates + scratch ----------------
            with nc.named_scope("phase1"), \
                 tc.tile_pool(name="p1w", bufs=3) as p1w, \
                 tc.tile_pool(name="p1sb", bufs=4) as p1sb, \
                 tc.tile_pool(name="p1eps", bufs=4, space="PSUM") as p1eps:

                def w_fetch(fb):
                    wth = p1w.tile([128, EC, 512], dt.float16, tag="wth",
                                   name=f"wth{fb}")
                    wtl = p1w.tile([128, EC, 512], dt.float16, tag="wtl",
                                   name=f"wtl{fb}")
                    nc.sync.dma_start(wth[:], wh_v[:, :, fb * 512:(fb + 1) * 512])
                    nc.sync.dma_start(wtl[:], wl_v[:, :, fb * 512:(fb + 1) * 512])
                    return wth, wtl

                preps = [w_fetch(0)]
                nc.sync.dma_start(xTh[:], xh_v)
                nc.sync.dma_start(xTl[:], xl_v)
                preps.append(w_fetch(1))
                nc.sync.dma_start(id16[:], id16_in)
                nc.sync.dma_start(bias_full[:], biasf_in)
                for fb in range(NFB):
                    wTh, wTl = preps.pop(0)
                    if fb + 2 < NFB:
                        preps.append(w_fetch(fb + 2))
                    for bt in range(NB):
                        eps = p1eps.tile([128, 512], dt.float32, tag="encps",
                                         name=f"encps{fb}_{bt}")
                        n_mm = 3 * EC
                        i = 0
                        for (xa, wa) in ((xTh, wTh), (xTh, wTl), (xTl, wTh)):
                            for ec in range(EC):
                                nc.tensor.matmul(
                                    eps[:],
                                    xa[:, ec, bt * 128:(bt + 1) * 128],
                                    wa[:, ec, :],
                                    start=(i == 0), stop=(i == n_mm - 1))
                                i += 1
                        ptile = p1sb.tile([128, 512], dt.float32, tag="ptile",
                                          name=f"ptile{fb}_{bt}")
                        nc.scalar.copy(ptile[:], eps[:])
                        nc.sync.dma_start(
                            proj_scr[bt * 128:(bt + 1) * 128, fb * 512:(fb + 1) * 512],
                            ptile[:])
                        for seg in range(2):
                            off = fb * 16 + seg * 8
                            nc.vector.max(cands[bt][:, off:off + 8],
                                          ptile[:, seg * 256:(seg + 1) * 256])
                        if fb == NFB - 1 and bt == 0:
                            taus.append(tau_find(bt))

            # ---------------- Phase 3: masked decoder ----------------
            def finalize_bt(bt, p4):
                """bias + row-normalize + store for one batch-tile."""
                rb = p4.tile([128, E], dt.float32, tag="rb", name=f"rb{bt}")
                nc.vector.tensor_tensor(rb[:], recon[:, bt, :], bias_full[:],
                                        op=Alu.add)
                sq = p4.tile([128, E], dt.float32, tag="sq", name=f"sq{bt}")
                nc.vector.tensor_tensor(sq[:], rb[:], rb[:], op=Alu.mult)
                ss = p4.tile([128, 1], dt.float32, tag="ss", name=f"ss{bt}")
                nc.vector.tensor_reduce(ss[:], sq[:], axis=mybir.AxisListType.X,
                                        op=Alu.add)
                nrm = p4.tile([128, 1], dt.float32, tag="nrm", name=f"nrm{bt}")
                nc.scalar.activation(nrm[:], ss[:], Act.Sqrt)
                nc.vector.tensor_scalar_max(nrm[:], nrm[:], 1e-12)
                inv = p4.tile([128, 1], dt.float32, tag="inv", name=f"inv{bt}")
                nc.vector.reciprocal(inv[:], nrm[:])
                ot = p4.tile([128, E], dt.float32, tag="ot", name=f"ot{bt}")
                nc.vector.tensor_scalar_mul(ot[:], rb[:], inv[:])
                nc.sync.dma_start(out_v[bt], ot[:])

            with nc.named_scope("phase3"), \
                 tc.tile_pool(name="p4sb", bufs=2) as p4, \
                 tc.tile_pool(name="p3d16", bufs=G + 1) as p3d16, \
                 tc.tile_pool(name="p3sb", bufs=8) as p3sb, \
                 tc.tile_pool(name="p3tps", bufs=4, space="PSUM") as p3tps, \
                 tc.tile_pool(name="p3dps", bufs=2, space="PSUM") as p3dps:
                for fbg in range(0, NFB, G):
                    d16s = []
                    for g in range(G):
                        d16 = p3d16.tile([128, 4, E], dt.float16, tag="d16",
                                         name=f"d16_{fbg + g}")
                        nc.sync.dma_start(d16[:], dec_v[fbg + g])
                        d16s.append(d16)
                    for bt in range(NB):
                        if fbg == 0 and bt > 0:
                            taus.append(tau_find(bt))
                        dps = [p3dps.tile([128, 384], dt.float32, tag=f"dps{eh}",
                                          name=f"dps{eh}_{fbg}_{bt}")
                               for eh in range(2)]
                        mTs = []
                        for g in range(G):
                            fb = fbg + g
                            stile = p3sb.tile([128, 512], dt.float32, tag="stile",
                                              name=f"stile{fb}_{bt}")
                            nc.sync.dma_start(
                                stile[:],
                                proj_scr[bt * 128:(bt + 1) * 128,
                                         fb * 512:(fb + 1) * 512])
                            mask01 = p3sb.tile([128, 512], dt.float32, tag="mask01",
                                               name=f"mask{fb}_{bt}")
                            nc.vector.tensor_scalar(mask01[:], stile[:],
                                                    taus[bt][:, 7:8], None,
                                                    op0=Alu.is_ge)
                            m16 = p3sb.tile([128, 512], dt.float16, tag="m16",
                                            name=f"m16_{fb}_{bt}")
                            nc.vector.tensor_tensor(m16[:], stile[:], mask01[:],
                                                    op=Alu.mult)
                            tps = p3tps.tile([128, 512], dt.float16, tag="tps",
                                             name=f"tps{fb}_{bt}")
                            for fs in range(4):
                                nc.tensor.transpose(tps[:, fs * 128:(fs + 1) * 128],
                                                    m16[:, fs * 128:(fs + 1) * 128],
                                                    id16[:])
                            mT = p3sb.tile([128, 512], dt.float16, tag="mT",
                                           name=f"mT{fb}_{bt}")
                            nc.scalar.copy(mT[:], tps[:])
                            mTs.append(mT)
                        for g in range(G):
                            for eh in range(2):
                                for fs in range(4):
                                    nc.tensor.matmul(
                                        dps[eh][:],
                                        mTs[g][:, fs * 128:(fs + 1) * 128],
                                        d16s[g][:, fs, eh * 384:(eh + 1) * 384],
                                        start=(g == 0 and fs == 0),
                                        stop=(g == G - 1 and fs == 3))
                        for eh in range(2):
                            nc.vector.tensor_tensor(
                                recon[:, bt, eh * 384:(eh + 1) * 384],
                                recon[:, bt, eh * 384:(eh + 1) * 384],
                                dps[eh][:], op=Alu.add)
                        if fbg == NFB - G:
                            finalize_bt(bt, p4)

    nc.finalize()
    return nc


_CACHE = {}


def _get_nc(NB, NFB):
    key = (NB, NFB)
    if key not in _CACHE:
        _CACHE[key] = build_kernel(NB, NFB)
    return _CACHE[key]


def run(embed, enc_bias, enc_weight, dec_lookup, NB=4, NFB=48, trace=False):
    B_loc = NB * 128
    wT = np.ascontiguousarray(enc_weight.T)  # [E, F] fp32
    wTh = wT.astype(np.float16)
    wTl = (wT - wTh.astype(np.float32)).astype(np.float16)
    dec16 = dec_lookup.astype(np.float16)
    biasf = np.broadcast_to(enc_bias.astype(np.float32), (128, E)).copy()
    eye16 = np.eye(128, dtype=np.float16)
    in_maps = []
    for c in range(N_CORES):
        xc = (embed[c * B_loc:(c + 1) * B_loc] - enc_bias).astype(np.float32)
        xT = np.ascontiguousarray(xc.T)  # [E, B_loc]
        xTh = xT.astype(np.float16)
        xTl = (xT - xTh.astype(np.float32)).astype(np.float16)
        in_maps.append({
            "xTh": xTh, "xTl": xTl, "wTh": wTh, "wTl": wTl, "dec16": dec16,
            "biasf": biasf, "ident16": eye16,
        })
    nc = _get_nc(NB, NFB)
    res = run_bass_kernel_spmd(nc, in_maps, list(range(N_CORES)), trace=trace)
    out = np.concatenate([res.results[c]["out"] for c in range(N_CORES)], axis=0)
    return out, res


def kernel(embed, enc_bias, enc_weight, dec_lookup):
    import time

    args = (np.asarray(embed, dtype=np.float32),
            np.asarray(enc_bias, dtype=np.float32),
            np.asarray(enc_weight, dtype=np.float32),
            np.asarray(dec_lookup, dtype=np.float32))
    # The axon-tunneled device pool occasionally hands out a wedged worker;
    # compile is cached, so retries are cheap.
    last_exc = None
    for attempt in range(3):
        try:
            out, _ = run(*args)
            return out
        except Exception as e:  # noqa: BLE001
            last_exc = e
            time.sleep(10.0)
    raise last_exc


# revision 11
# speedup vs baseline: 1.1902x; 1.1902x over previous
"""TopK sparse autoencoder forward pass on 8 Trainium2 NeuronCores — v3.

Baseline structure (3-pass fp16 hi/lo encoder, threshold top-64 via per-256
chunk top-8 candidates + exact tau, dense masked fp16 decode) with all input
marshaling moved to the host: enc_weight arrives pre-transposed and hi/lo
fp16-split, x arrives bias-subtracted/transposed/split, dec_lookup arrives
fp16. The PE runs only encoder matmuls + mask transposes + decode matmuls.
"""

import sys

sys.path.insert(0, "/opt/trn_rl_repo")

import numpy as np  # noqa: E402

import concourse.bacc as bacc  # noqa: E402
import concourse.mybir as mybir  # noqa: E402
import concourse.tile as tile  # noqa: E402
from concourse.bass_utils import run_bass_kernel_spmd  # noqa: E402

dt = mybir.dt
Alu = mybir.AluOpType
Act = mybir.ActivationFunctionType

N_CORES = 8
E = 768
EC = E // 128  # 6 e-chunks
NEG_FILL = -1e30
G = 6  # decoder f-block accumulation group


def build_kernel(NB=4, NFB=48):
    """NB: batch tiles of 128 rows per core; NFB: feature blocks of 512."""
    B_loc = NB * 128
    F = NFB * 512
    G = min(globals()["G"], NFB)
    NCAND = NFB * 2 * 8  # top-8 per 256-feat chunk

    nc = bacc.Bacc("TRN2", target_bir_lowering=False, debug=False,
                   num_devices=N_CORES)
    xh_in = nc.dram_tensor("xTh", [E, B_loc], dt.float16, kind="ExternalInput").ap()
    xl_in = nc.dram_tensor("xTl", [E, B_loc], dt.float16, kind="ExternalInput").ap()
    wh_in = nc.dram_tensor("wTh", [E, F], dt.float16, kind="ExternalInput").ap()
    wl_in = nc.dram_tensor("wTl", [E, F], dt.float16, kind="ExternalInput").ap()
    dec_in = nc.dram_tensor("dec16", [F, E], dt.float16, kind="ExternalInput").ap()
    biasf_in = nc.dram_tensor("biasf", [128, E], dt.float32, kind="ExternalInput").ap()
    id16_in = nc.dram_tensor("ident16", [128, 128], dt.float16, kind="ExternalInput").ap()
    out_ext = nc.dram_tensor("out", [B_loc, E], dt.float32, kind="ExternalOutput").ap()
    proj_scr = nc.dram_tensor("proj_scr", [B_loc, F], dt.float32).ap()

    xh_v = xh_in.rearrange("(ec p) b -> p ec b", p=128)
    xl_v = xl_in.rearrange("(ec p) b -> p ec b", p=128)
    wh_v = wh_in.rearrange("(ec p) f -> p ec f", p=128)
    wl_v = wl_in.rearrange("(ec p) f -> p ec f", p=128)
    dec_v = dec_in.rearrange("(blk t p) e -> blk p t e", p=128, t=4)
    out_v = out_ext.rearrange("(bt p) e -> bt p e", p=128)

    with tile.TileContext(nc) as tc:
        with tc.tile_pool(name="persist", bufs=1) as pp:
            xTh = pp.tile([128, EC, B_loc], dt.float16, tag="xTh")
            xTl = pp.tile([128, EC, B_loc], dt.float16, tag="xTl")
            id16 = pp.tile([128, 128], dt.float16, tag="id16")
            bias_full = pp.tile([128, E], dt.float32, tag="bias_full")
            # candidates per batch-tile
            cands = [pp.tile([128, NCAND], dt.float32, tag=f"cand{bt}",
                             name=f"cand{bt}") for bt in range(NB)]
            # recon accumulator
            recon = pp.tile([128, NB, E], dt.float32, tag="recon")
            nc.vector.memset(recon[:], 0.0)
            taus = []

            def tau_find(bt):
                """exact 64th-largest of bt's candidates (destroys cands[bt])."""
                m8 = None
                for r in range(8):
                    m8 = pp.tile([128, 8], dt.float32, tag=f"m8_{bt}_{r}",
                                 name=f"m8_{bt}_{r}")
                    nc.vector.max(m8[:], cands[bt][:])
                    if r < 7:
                        nc.vector.match_replace(cands[bt][:], m8[:], cands[bt][:],
                                                NEG_FILL)
                return m8

            # ---------------- Phase 1: encoder + candidates + scratch ----------------
            with nc.named_scope("phase1"), \
                 tc.tile_pool(name="p1w", bufs=3) as p1w, \
                 tc.tile_pool(name="p1sb", bufs=4) as p1sb, \
                 tc.tile_pool(name="p1eps", bufs=4, space="PSUM") as p1eps:

                def w_fetch(fb):
                    wth = p1w.tile([128, EC, 512], dt.float16, tag="wth",
                                   name=f"wth{fb}")
                    wtl = p1w.tile([128, EC, 512], dt.float16, tag="wtl",
                                   name=f"wtl{fb}")
                    nc.sync.dma_start(wth[:], wh_v[:, :, fb * 512:(fb + 1) * 512])
                    nc.sync.dma_start(wtl[:], wl_v[:, :, fb * 512:(fb + 1) * 512])
                    return wth, wtl

                preps = [w_fetch(0)]
                nc.sync.dma_start(xTh[:], xh_v)
                nc.sync.dma_start(xTl[:], xl_v)
                preps.append(w_fetch(1))
                nc.sync.dma_start(id16[:], id16_in)
                nc.sync.dma_start(bias_full[:], biasf_in)
                for fb in range(NFB):
                    wTh, wTl = preps.pop(0)
                    if fb + 2 < NFB:
                        preps.append(w_fetch(fb + 2))
                    for bt in range(NB):
                        eps = p1eps.tile([128, 512], dt.float32, tag="encps",
                                         name=f"encps{fb}_{bt}")
                        n_mm = 3 * EC
                        i = 0
                        for (xa, wa) in ((xTh, wTh), (xTh, wTl), (xTl, wTh)):
                            for ec in range(EC):
                                nc.tensor.matmul(
                                    eps[:],
                                    xa[:, ec, bt * 128:(bt + 1) * 128],
                                    wa[:, ec, :],
                                    start=(i == 0), stop=(i == n_mm - 1))
                                i += 1
                        ptile = p1sb.tile([128, 512], dt.float32, tag="ptile",
                                          name=f"ptile{fb}_{bt}")
                        nc.scalar.copy(ptile[:], eps[:])
                        nc.sync.dma_start(
                            proj_scr[bt * 128:(bt + 1) * 128, fb * 512:(fb + 1) * 512],
                            ptile[:])
                        for seg in range(2):
                            off = fb * 16 + seg * 8
                            nc.vector.max(cands[bt][:, off:off + 8],
                                          ptile[:, seg * 256:(seg + 1) * 256])
                        if fb == NFB - 1 and bt == 0:
                            taus.append(tau_find(bt))

            # ---------------- Phase 3: masked decoder ----------------
            def finalize_bt(bt, p4):
                """bias + row-normalize + store for one batch-tile."""
                rb = p4.tile([128, E], dt.float32, tag="rb", name=f"rb{bt}")
                nc.vector.tensor_tensor(rb[:], recon[:, bt, :], bias_full[:],
                                        op=Alu.add)
                sq = p4.tile([128, E], dt.float32, tag="sq", name=f"sq{bt}")
                nc.vector.tensor_tensor(sq[:], rb[:], rb[:], op=Alu.mult)
                ss = p4.tile([128, 1], dt.float32, tag="ss", name=f"ss{bt}")
                nc.vector.tensor_reduce(ss[:], sq[:], axis=mybir.AxisListType.X,
                                        op=Alu.add)
                nrm = p4.tile([128, 1], dt.float32, tag="nrm", name=f"nrm{bt}")
                nc.scalar.activation(nrm[:], ss[:], Act.Sqrt)
                nc.vector.tensor_scalar_max(nrm[:], nrm[:], 1e-12)
                inv = p4.tile([128, 1], dt.float32, tag="inv", name=f"inv{bt}")
                nc.vector.reciprocal(inv[:], nrm[:])
                ot = p4.tile([128, E], dt.float32, tag="ot", name=f"ot{bt}")
                nc.vector.tensor_scalar_mul(ot[:], rb[:], inv[:])
                nc.sync.dma_start(out_v[bt], ot[:])

            with nc.named_scope("phase3"), \
                 tc.tile_pool(name="p4sb", bufs=2) as p4, \
                 tc.tile_pool(name="p3d16", bufs=G + 1) as p3d16, \
                 tc.tile_pool(name="p3sb", bufs=8) as p3sb, \
                 tc.tile_pool(name="p3tps", bufs=4, space="PSUM") as p3tps, \
                 tc.tile_pool(name="p3dps", bufs=2, space="PSUM") as p3dps:
                for fbg in range(0, NFB, G):
                    d16s = []
                    for g in range(G):
                        d16 = p3d16.tile([128, 4, E], dt.float16, tag="d16",
                                         name=f"d16_{fbg + g}")
                        nc.sync.dma_start(d16[:], dec_v[fbg + g])
                        d16s.append(d16)
                    for bt in range(NB):
                        if fbg == 0 and bt > 0:
                            taus.append(tau_find(bt))
                        dps = [p3dps.tile([128, 384], dt.float32, tag=f"dps{eh}",
                                          name=f"dps{eh}_{fbg}_{bt}")
                               for eh in range(2)]
                        mTs = []
                        for g in range(G):
                            fb = fbg + g
                            stile = p3sb.tile([128, 512], dt.float32, tag="stile",
                                              name=f"stile{fb}_{bt}")
                            nc.sync.dma_start(
                                stile[:],
                                proj_scr[bt * 128:(bt + 1) * 128,
                                         fb * 512:(fb + 1) * 512])
                            mask01 = p3sb.tile([128, 512], dt.float32, tag="mask01",
                                               name=f"mask{fb}_{bt}")
                            nc.vector.tensor_scalar(mask01[:], stile[:],
                                                    taus[bt][:, 7:8], None,
                                                    op0=Alu.is_ge)
                            m16 = p3sb.tile([128, 512], dt.float16, tag="m16",
                                            name=f"m16_{fb}_{bt}")
                            nc.vector.tensor_tensor(m16[:], stile[:], mask01[:],
                                                    op=Alu.mult)
                            tps = p3tps.tile([128, 512], dt.float16, tag="tps",
                                             name=f"tps{fb}_{bt}")
                            for fs in range(4):
                                nc.tensor.transpose(tps[:, fs * 128:(fs + 1) * 128],
                                                    m16[:, fs * 128:(fs + 1) * 128],
                                                    id16[:])
                            mT = p3sb.tile([128, 512], dt.float16, tag="mT",
                                           name=f"mT{fb}_{bt}")
                            nc.scalar.copy(mT[:], tps[:])
                            mTs.append(mT)
                        for g in range(G):
                            for eh in range(2):
                                for fs in range(4):
                                    nc.tensor.matmul(
                                        dps[eh][:],
                                        mTs[g][:, fs * 128:(fs + 1) * 128],
                                        d16s[g][:, fs, eh * 384:(eh + 1) * 384],
                                        start=(g == 0 and fs == 0),
                                        stop=(g == G - 1 and fs == 3))
                        for eh in range(2):
                            nc.vector.tensor_tensor(
                                recon[:, bt, eh * 384:(eh + 1) * 384],
                                recon[:, bt, eh * 384:(eh + 1) * 384],
                                dps[eh][:], op=Alu.add)
                        if fbg == NFB - G:
                            finalize_bt(bt, p4)

    nc.finalize()
    return nc


_CACHE = {}


def _get_nc(NB, NFB):
    key = (NB, NFB)
    if key not in _CACHE:
        _CACHE[key] = build_kernel(NB, NFB)
    return _CACHE[key]


def run(embed, enc_bias, enc_weight, dec_lookup, NB=4, NFB=48, trace=False):
    B_loc = NB * 128
    wT = np.ascontiguousarray(enc_weight.T)  # [E, F] fp32
    wTh = wT.astype(np.float16)
    wTl = (wT - wTh.astype(np.float32)).astype(np.float16)
    dec16 = dec_lookup.astype(np.float16)
    biasf = np.broadcast_to(enc_bias.astype(np.float32), (128, E)).copy()
    eye16 = np.eye(128, dtype=np.float16)
    in_maps = []
    for c in range(N_CORES):
        xc = (embed[c * B_loc:(c + 1) * B_loc] - enc_bias).astype(np.float32)
        xT = np.ascontiguousarray(xc.T)  # [E, B_loc]
        xTh = xT.astype(np.float16)
        xTl = (xT - xTh.astype(np.float32)).astype(np.float16)
        in_maps.append({
            "xTh": xTh, "xTl": xTl, "wTh": wTh, "wTl": wTl, "dec16": dec16,
            "biasf": biasf, "ident16": eye16,
        })
    nc = _get_nc(NB, NFB)
    res = run_bass_kernel_spmd(nc, in_maps, list(range(N_CORES)), trace=trace)
    out = np.concatenate([res.results[c]["out"] for c in range(N_CORES)], axis=0)
    return out, res


def _spot_check(out, embed, enc_bias, enc_weight, dec_lookup, n=32):
    """Verify a sample of rows on host; a wedged worker can return silently
    corrupted results (observed once: rel err 0.27 on a byte-identical
    build). Broad corruption (a bad core/tile) is caught with high
    probability by 32 spread samples."""
    rows = np.linspace(0, out.shape[0] - 1, n).astype(int)
    for r in rows:
        x = (embed[r] - enc_bias).astype(np.float32)
        proj = enc_weight @ x
        idx = np.argpartition(-proj, 64)[:64]
        recon = proj[idx] @ dec_lookup[idx] + enc_bias
        nn = max(np.sqrt((recon * recon).sum()), 1e-12)
        ref = recon / nn
        if not np.isfinite(out[r]).all() or np.abs(out[r] - ref).max() > 2e-2:
            return False
    return True


def kernel(embed, enc_bias, enc_weight, dec_lookup):
    import time

    args = (np.asarray(embed, dtype=np.float32),
            np.asarray(enc_bias, dtype=np.float32),
            np.asarray(enc_weight, dtype=np.float32),
            np.asarray(dec_lookup, dtype=np.float32))
    # The axon-tunneled device pool occasionally hands out a wedged worker;
    # compile is cached, so retries are cheap. Retry on exceptions AND on
    # silently corrupted outputs (host spot-check).
    last_exc = None
    out = None
    for attempt in range(4):
        try:
            out, _ = run(*args)
        except Exception as e:  # noqa: BLE001
            last_exc = e
            time.sleep(10.0)
            continue
        if _spot_check(out, *args):
            return out
        time.sleep(5.0)
    if out is not None:
        return out
    raise last_exc
